# revision 15
# baseline (speedup 1.0000x reference)
# GAU (Gated Attention Unit) kernel for Trainium2, 8 NeuronCores.
#
# Sharding: batch x sequence-half. Core c handles batch c//2, sequence half
# c%2 (2048 "own" query rows). Each core receives its batch's full x with the
# own rows rotated to the front -- attention here (relu^2, no softmax, no mask)
# is permutation-invariant over the key/value index j, so one SPMD NEFF serves
# all 8 cores with no collectives and no runtime branching.
#
# Per-core pipeline (bf16 matmuls, fp32 PSUM accumulation):
#   A: x [4096,512] f32 -> gpsimd casting DMA -> bf16 SBUF staging ->
#      PE transpose (bf16, 1 cyc/row) -> xT [d,n] bf16
#      (xbar DMA transpose was faster on paper but intermittently wedged the
#      device when all 8 cores ran it concurrently with other DMA traffic)
#   B: qkT = w_qk.T @ xT; kT/qT via per-partition affine (attn scale folded
#      into gamma/beta on host)
#   C: v = xT.T @ w_v (natural [j,h]); gateT = w_g.T @ xT ([h,i])
#   D: per 512-row i-block: simT[j,i] = kT.T @ qT -> relu (ACT) -> square (DVE)
#   E: oT[h,i] = sum_j v[j,:].T @ attnT; multiply by gateT
#   F: out[i,:] = og.T @ w_out + x (residual re-read in f32); DMA out
#
# b_hidden is assumed zero (it is, for this problem's fixed setup_inputs);
# b_out is applied exactly on the host (it is also zero).

import numpy as np
import ml_dtypes

import concourse.bass as bass
import concourse.bacc as bacc
import concourse.mybir as mybir
import concourse.tile as tile
from concourse.masks import make_identity
from concourse.bass_utils import run_bass_kernel_spmd

F32 = mybir.dt.float32
BF16 = mybir.dt.bfloat16
AF = mybir.ActivationFunctionType

B, N, D, E, H = 4, 4096, 512, 128, 1024
NOWN = N // 2          # own rows per core
NB = N // 128          # 32 row tiles
DC = D // 128          # 4 contraction chunks
HB = H // 128          # 8 hidden chunks
JC = N // 128          # 32 key chunks
IBLK = NOWN // 512     # 4 i-blocks of 512

_CACHE = {}


def build_nc():
    nc = bacc.Bacc("TRN2", target_bir_lowering=False)
    x_in = nc.dram_tensor("x_in", [N, D], F32, kind="ExternalInput")
    w_h = nc.dram_tensor("w_h", [D, 2 * H], BF16, kind="ExternalInput")
    w_qk = nc.dram_tensor("w_qk", [D, E], BF16, kind="ExternalInput")
    w_o = nc.dram_tensor("w_o", [H, D], BF16, kind="ExternalInput")
    qg = nc.dram_tensor("qg", [E, 1], F32, kind="ExternalInput")
    qb = nc.dram_tensor("qb", [E, 1], F32, kind="ExternalInput")
    kg = nc.dram_tensor("kg", [E, 1], F32, kind="ExternalInput")
    kb = nc.dram_tensor("kb", [E, 1], F32, kind="ExternalInput")
    out = nc.dram_tensor("out", [NOWN, D], F32, kind="ExternalOutput")

    with tile.TileContext(nc) as tc:
        with tc.tile_pool(name="persist", bufs=1) as per:
            # small per-partition affine params (scalar queue; tiny)
            qg_sb = per.tile([E, 1], F32, name="qg_sb", tag="qg_sb")
            qb_sb = per.tile([E, 1], F32, name="qb_sb", tag="qb_sb")
            kg_sb = per.tile([E, 1], F32, name="kg_sb", tag="kg_sb")
            kb_sb = per.tile([E, 1], F32, name="kb_sb", tag="kb_sb")
            nc.scalar.dma_start(qg_sb, qg[:, :])
            nc.scalar.dma_start(qb_sb, qb[:, :])
            nc.scalar.dma_start(kg_sb, kg[:, :])
            nc.scalar.dma_start(kb_sb, kb[:, :])

            kT = per.tile([128, N], BF16, name="kT", tag="kT")
            qT = per.tile([128, NOWN], BF16, name="qT", tag="qT")
            # v natural [j, h]: tile jc holds rows jc*128..+128, cols jc*H..(jc+1)*H
            v_big = per.tile([128, JC * H], BF16, name="v_big", tag="v_big")
            # gateT [h, i]: chunk hb at cols hb*NOWN..+NOWN
            gT_big = per.tile([128, HB * NOWN], BF16, name="gT_big", tag="gT_big")
            wo_sb = per.tile([128, HB * D], BF16, name="wo_sb", tag="wo_sb")
            wqk_sb = per.tile([128, DC * E], BF16, name="wqk_sb", tag="wqk_sb")
            # merged 3D weight DMAs on the scalar HWDGE queue (parallel to
            # the sync queue that carries the transpose stream)
            nc.scalar.dma_start(
                wo_sb.rearrange("p (hc d) -> p hc d", hc=HB),
                w_o[:, :].rearrange("(hc p) d -> p hc d", p=128),
            )
            nc.scalar.dma_start(
                wqk_sb.rearrange("p (dc e) -> p dc e", dc=DC),
                w_qk[:, :].rearrange("(dc p) e -> p dc e", p=128),
            )

            ident = per.tile([128, 128], BF16, name="ident", tag="ident")
            make_identity(nc, ident)

            # ---------------- phases A-C ----------------
            with tc.tile_pool(name="abc_sb", bufs=1) as abc, \
                 tc.tile_pool(name="xs_pool", bufs=3) as xs_pool, \
                 tc.tile_pool(name="tp_ps", bufs=4, space="PSUM") as tp_ps, \
                 tc.tile_pool(name="mm_ps", bufs=4, space="PSUM") as mm_ps:
                xT = abc.tile([128, DC * N], BF16, name="xT", tag="xT")
                wh_sb = abc.tile([128, DC * 2 * H], BF16, name="wh_sb", tag="wh_sb")
                nc.scalar.dma_start(
                    wh_sb.rearrange("p (dc h) -> p dc h", dc=DC),
                    w_h[:, :].rearrange("(dc p) h -> p dc h", p=128),
                )

                # A: load x as bf16 (even groups: gpsimd casting DMA; odd
                # groups: sync f32 DMA + DVE cast -- two parallel DMA queues),
                # then bf16 PE transposes into xT
                for g in range(NB // 4):
                    xsb = xs_pool.tile([128, 4 * D], BF16, name="xsb",
                                       tag="xsb", bufs=2)
                    src = x_in[g * 512:(g + 1) * 512, :].rearrange(
                        "(t p) d -> p t d", p=128
                    )
                    if g % 2 == 0:
                        nc.gpsimd.dma_start(
                            xsb.rearrange("p (t d) -> p t d", t=4), src
                        )
                    else:
                        xf = xs_pool.tile([128, 4 * D], F32, name="xf",
                                          tag="xf", bufs=2)
                        nc.sync.dma_start(
                            xf.rearrange("p (t d) -> p t d", t=4), src
                        )
                        nc.vector.tensor_copy(xsb, xf)
                    for t in range(4):
                        nb = g * 4 + t
                        for dc in range(DC):
                            pst = tp_ps.tile([128, 128], BF16, name="pst", tag="pst")
                            nc.tensor.transpose(
                                pst, xsb[:, t * D + dc * 128: t * D + (dc + 1) * 128],
                                ident,
                            )
                            dst = xT[:, dc * N + nb * 128: dc * N + (nb + 1) * 128]
                            if (nb * DC + dc) % 2 == 0:
                                nc.vector.tensor_copy(dst, pst)
                            else:
                                nc.scalar.copy(dst, pst)

                # B: qkT -> kT (all), qT (own half)
                for nblk in range(N // 512):
                    ps = mm_ps.tile([128, 512], F32, name="qk_ps", tag="mmps")
                    for dc in range(DC):
                        nc.tensor.matmul(
                            ps,
                            wqk_sb[:, dc * E:(dc + 1) * E],
                            xT[:, dc * N + nblk * 512: dc * N + (nblk + 1) * 512],
                            start=(dc == 0),
                            stop=(dc == DC - 1),
                        )
                    nc.scalar.activation(
                        kT[:, nblk * 512:(nblk + 1) * 512], ps, AF.Identity,
                        bias=kb_sb, scale=kg_sb,
                    )
                    if nblk < NOWN // 512:
                        nc.scalar.activation(
                            qT[:, nblk * 512:(nblk + 1) * 512], ps, AF.Identity,
                            bias=qb_sb, scale=qg_sb,
                        )

                # C: v (natural layout)
                for jb in range(NB):
                    for hh in range(2):
                        ps = mm_ps.tile([128, 512], F32, name="v_ps", tag="mmps")
                        for dc in range(DC):
                            nc.tensor.matmul(
                                ps,
                                xT[:, dc * N + jb * 128: dc * N + (jb + 1) * 128],
                                wh_sb[:, dc * 2 * H + hh * 512: dc * 2 * H + (hh + 1) * 512],
                                start=(dc == 0),
                                stop=(dc == DC - 1),
                            )
                        dst = v_big[:, jb * H + hh * 512: jb * H + (hh + 1) * 512]
                        if (jb * 2 + hh) % 2 == 0:
                            nc.vector.tensor_copy(dst, ps)
                        else:
                            nc.scalar.copy(dst, ps)

                # C: gateT
                for hb in range(HB):
                    for ib in range(IBLK):
                        ps = mm_ps.tile([128, 512], F32, name="g_ps", tag="mmps")
                        for dc in range(DC):
                            nc.tensor.matmul(
                                ps,
                                wh_sb[:, dc * 2 * H + H + hb * 128: dc * 2 * H + H + (hb + 1) * 128],
                                xT[:, dc * N + ib * 512: dc * N + (ib + 1) * 512],
                                start=(dc == 0),
                                stop=(dc == DC - 1),
                            )
                        dst = gT_big[:, hb * NOWN + ib * 512: hb * NOWN + (ib + 1) * 512]
                        if (hb * IBLK + ib) % 2 == 0:
                            nc.vector.tensor_copy(dst, ps)
                        else:
                            nc.scalar.copy(dst, ps)

            # ---------------- phases D-F ----------------
            with tc.tile_pool(name="at_pool", bufs=36) as at_pool, \
                 tc.tile_pool(name="og_pool", bufs=10) as og_pool, \
                 tc.tile_pool(name="st_pool", bufs=2) as st_pool, \
                 tc.tile_pool(name="sim_ps", bufs=3, space="PSUM") as sim_ps, \
                 tc.tile_pool(name="ot_ps", bufs=2, space="PSUM") as ot_ps, \
                 tc.tile_pool(name="out_ps", bufs=2, space="PSUM") as out_ps:
                for blk in range(IBLK):
                    i0 = blk * 512
                    # D: simT -> attnT = relu(sim)^2, bf16
                    attn = []
                    for jc in range(JC):
                        sps = sim_ps.tile([128, 512], F32, name="sps", tag="sps")
                        nc.tensor.matmul(
                            sps,
                            kT[:, jc * 128:(jc + 1) * 128],
                            qT[:, i0:i0 + 512],
                            start=True, stop=True,
                        )
                        at = at_pool.tile([128, 512], BF16, name="at", tag="at")
                        nc.scalar.activation(at, sps, AF.Relu)
                        nc.vector.tensor_mul(at, at, at)
                        attn.append(at)

                    # E: oT chains (hc outer), gate multiply
                    og_tiles = []
                    for hc in range(HB):
                        ops = ot_ps.tile([128, 512], F32, name="ops", tag="ops")
                        for jc in range(JC):
                            nc.tensor.matmul(
                                ops,
                                v_big[:, jc * H + hc * 128: jc * H + (hc + 1) * 128],
                                attn[jc],
                                start=(jc == 0),
                                stop=(jc == JC - 1),
                            )
                        og = og_pool.tile([128, 512], BF16, name="og", tag="og")
                        nc.vector.tensor_mul(
                            og, ops, gT_big[:, hc * NOWN + i0: hc * NOWN + i0 + 512]
                        )
                        og_tiles.append(og)

                    # F: final projection + residual (merged per-block DMAs)
                    res = st_pool.tile([128, 4 * D], F32, name="res", tag="res")
                    nc.sync.dma_start(
                        res.rearrange("p (ic d) -> p ic d", ic=4),
                        x_in[i0:i0 + 512, :].rearrange("(ic p) d -> p ic d", p=128),
                    )
                    for ic in range(4):
                        ups = out_ps.tile([128, 512], F32, name="ups", tag="ups")
                        for hc in range(HB):
                            nc.tensor.matmul(
                                ups,
                                og_tiles[hc][:, ic * 128:(ic + 1) * 128],
                                wo_sb[:, hc * D:(hc + 1) * D],
                                start=(hc == 0),
                                stop=(hc == HB - 1),
                            )
                        osb = st_pool.tile([128, 512], F32, name="osb",
                                           tag="osb", bufs=6)
                        nc.vector.tensor_add(
                            osb, ups, res[:, ic * D:(ic + 1) * D]
                        )
                        row0 = i0 + ic * 128
                        nc.sync.dma_start(out[row0:row0 + 128, :], osb)
    nc.compile()
    return nc


def _numpy_fallback(x, w_hidden, b_hidden, w_qk, q_gamma, q_beta,
                    k_gamma, k_beta, w_out, b_out):
    scale = E ** -0.5
    hid = np.einsum("bnd,dh->bnh", x, w_hidden) + b_hidden
    v, gate = np.split(hid, 2, axis=-1)
    qk = np.einsum("bnd,de->bne", x, w_qk)
    q = qk * q_gamma + q_beta
    k = qk * k_gamma + k_beta
    sim = np.einsum("bid,bjd->bij", q, k) * scale
    attn = np.square(np.maximum(sim, 0.0))
    o = np.einsum("bij,bjh->bih", attn, v) * gate
    o = np.einsum("bnh,hd->bnd", o, w_out) + b_out
    return (o + x).astype(x.dtype)


def _run(inputs, trace=False):
    x = np.asarray(inputs["x"], dtype=np.float32)
    b_hidden = np.asarray(inputs["b_hidden"], dtype=np.float32)
    b_out = np.asarray(inputs["b_out"], dtype=np.float32)
    if np.any(b_hidden):
        # device kernel folds no hidden bias; this problem's setup has zeros
        return _numpy_fallback(**{k: np.asarray(v) for k, v in inputs.items()}), None

    bf16 = ml_dtypes.bfloat16
    s4 = float(E) ** -0.25  # sqrt of attention scale, folded into q and k
    wh_bf = np.ascontiguousarray(np.asarray(inputs["w_hidden"]).astype(bf16))
    wqk_bf = np.ascontiguousarray(np.asarray(inputs["w_qk"]).astype(bf16))
    wo_bf = np.ascontiguousarray(np.asarray(inputs["w_out"]).astype(bf16))
    qg2 = np.ascontiguousarray((np.asarray(inputs["q_gamma"]) * s4).astype(np.float32).reshape(E, 1))
    qb2 = np.ascontiguousarray((np.asarray(inputs["q_beta"]) * s4).astype(np.float32).reshape(E, 1))
    kg2 = np.ascontiguousarray((np.asarray(inputs["k_gamma"]) * s4).astype(np.float32).reshape(E, 1))
    kb2 = np.ascontiguousarray((np.asarray(inputs["k_beta"]) * s4).astype(np.float32).reshape(E, 1))

    if "nc" not in _CACHE:
        _CACHE["nc"] = build_nc()
    nc = _CACHE["nc"]

    in_maps = []
    for c in range(8):
        bi, hi = c // 2, c % 2
        xb = x[bi]
        xp = xb if hi == 0 else np.concatenate([xb[NOWN:], xb[:NOWN]], axis=0)
        in_maps.append({
            "x_in": np.ascontiguousarray(xp),
            "w_h": wh_bf, "w_qk": wqk_bf, "w_o": wo_bf,
            "qg": qg2, "qb": qb2, "kg": kg2, "kb": kb2,
        })

    kw = {}
    if trace:
        kw = dict(trace=True, trace_cores=[0])
    r = run_bass_kernel_spmd(nc, in_maps, core_ids=list(range(8)), **kw)

    out = np.empty((B, N, D), dtype=np.float32)
    for c in range(8):
        bi, hi = c // 2, c % 2
        out[bi, hi * NOWN:(hi + 1) * NOWN] = r.results[c]["out"]
    if np.any(b_out):
        out += b_out
    return out, r


def kernel(**inputs):
    out, _ = _run(inputs)
    return out


# revision 18
# speedup vs baseline: 1.0087x; 1.0087x over previous
# GAU (Gated Attention Unit) kernel for Trainium2, 8 NeuronCores.
#
# Sharding: batch x sequence-half. Core c handles batch c//2, sequence half
# c%2 (2048 "own" query rows). Each core receives its batch's full x with the
# own rows rotated to the front -- attention here (relu^2, no softmax, no mask)
# is permutation-invariant over the key/value index j, so one SPMD NEFF serves
# all 8 cores with no collectives and no runtime branching.
#
# Per-core pipeline (bf16 matmuls, fp32 PSUM accumulation):
#   A: x [4096,512] f32 -> gpsimd casting DMA -> bf16 SBUF staging ->
#      PE transpose (bf16, 1 cyc/row) -> xT [d,n] bf16
#      (xbar DMA transpose was faster on paper but intermittently wedged the
#      device when all 8 cores ran it concurrently with other DMA traffic)
#   B: qkT = w_qk.T @ xT; kT/qT via per-partition affine (attn scale folded
#      into gamma/beta on host)
#   C: v = xT.T @ w_v (natural [j,h]); gateT = w_g.T @ xT ([h,i])
#   D: per 512-row i-block: simT[j,i] = kT.T @ qT -> relu (ACT) -> square (DVE)
#   E: oT[h,i] = sum_j v[j,:].T @ attnT; multiply by gateT
#   F: out[i,:] = og.T @ w_out + x (residual re-read in f32); DMA out
#
# b_hidden is assumed zero (it is, for this problem's fixed setup_inputs);
# b_out is applied exactly on the host (it is also zero).

import numpy as np
import ml_dtypes

import concourse.bass as bass
import concourse.bacc as bacc
import concourse.mybir as mybir
import concourse.tile as tile
from concourse.masks import make_identity
from concourse.bass_utils import run_bass_kernel_spmd

F32 = mybir.dt.float32
BF16 = mybir.dt.bfloat16
AF = mybir.ActivationFunctionType

B, N, D, E, H = 4, 4096, 512, 128, 1024
NOWN = N // 2          # own rows per core
NB = N // 128          # 32 row tiles
DC = D // 128          # 4 contraction chunks
HB = H // 128          # 8 hidden chunks
JC = N // 128          # 32 key chunks
IBLK = NOWN // 512     # 4 i-blocks of 512

_CACHE = {}


def build_nc():
    nc = bacc.Bacc("TRN2", target_bir_lowering=False)
    x_in = nc.dram_tensor("x_in", [N, D], F32, kind="ExternalInput")
    w_h = nc.dram_tensor("w_h", [D, 2 * H], BF16, kind="ExternalInput")
    w_qk = nc.dram_tensor("w_qk", [D, E], BF16, kind="ExternalInput")
    w_o = nc.dram_tensor("w_o", [H, D], BF16, kind="ExternalInput")
    qg = nc.dram_tensor("qg", [E, 1], F32, kind="ExternalInput")
    qb = nc.dram_tensor("qb", [E, 1], F32, kind="ExternalInput")
    kg = nc.dram_tensor("kg", [E, 1], F32, kind="ExternalInput")
    kb = nc.dram_tensor("kb", [E, 1], F32, kind="ExternalInput")
    out = nc.dram_tensor("out", [NOWN, D], F32, kind="ExternalOutput")

    with tile.TileContext(nc) as tc:
        with tc.tile_pool(name="persist", bufs=1) as per:
            # small per-partition affine params (scalar queue; tiny)
            qg_sb = per.tile([E, 1], F32, name="qg_sb", tag="qg_sb")
            qb_sb = per.tile([E, 1], F32, name="qb_sb", tag="qb_sb")
            kg_sb = per.tile([E, 1], F32, name="kg_sb", tag="kg_sb")
            kb_sb = per.tile([E, 1], F32, name="kb_sb", tag="kb_sb")
            nc.scalar.dma_start(qg_sb, qg[:, :])
            nc.scalar.dma_start(qb_sb, qb[:, :])
            nc.scalar.dma_start(kg_sb, kg[:, :])
            nc.scalar.dma_start(kb_sb, kb[:, :])

            kT = per.tile([128, N], BF16, name="kT", tag="kT")
            qT = per.tile([128, NOWN], BF16, name="qT", tag="qT")
            # v natural [j, h]: tile jc holds rows jc*128..+128, cols jc*H..(jc+1)*H
            v_big = per.tile([128, JC * H], BF16, name="v_big", tag="v_big")
            # gateT [h, i]: chunk hb at cols hb*NOWN..+NOWN
            gT_big = per.tile([128, HB * NOWN], BF16, name="gT_big", tag="gT_big")
            wo_sb = per.tile([128, HB * D], BF16, name="wo_sb", tag="wo_sb")
            wqk_sb = per.tile([128, DC * E], BF16, name="wqk_sb", tag="wqk_sb")
            # weights ride the scalar HWDGE queue, parallel to the x loads
            # on the sync + gpsimd queues; wqk/wh first (needed earliest)
            nc.scalar.dma_start(
                wqk_sb.rearrange("p (dc e) -> p dc e", dc=DC),
                w_qk[:, :].rearrange("(dc p) e -> p dc e", p=128),
            )

            ident = per.tile([128, 128], BF16, name="ident", tag="ident")
            make_identity(nc, ident)

            # ---------------- phases A-C ----------------
            with tc.tile_pool(name="abc_sb", bufs=1) as abc, \
                 tc.tile_pool(name="xs_pool", bufs=3) as xs_pool, \
                 tc.tile_pool(name="tp_ps", bufs=4, space="PSUM") as tp_ps, \
                 tc.tile_pool(name="mm_ps", bufs=4, space="PSUM") as mm_ps:
                xT = abc.tile([128, DC * N], BF16, name="xT", tag="xT")
                wh_sb = abc.tile([128, DC * 2 * H], BF16, name="wh_sb", tag="wh_sb")
                for dc in range(DC):
                    nc.scalar.dma_start(
                        wh_sb[:, dc * 2 * H:(dc + 1) * 2 * H],
                        w_h[dc * 128:(dc + 1) * 128, :],
                    )
                nc.scalar.dma_start(
                    wo_sb.rearrange("p (hc d) -> p hc d", hc=HB),
                    w_o[:, :].rearrange("(hc p) d -> p hc d", p=128),
                )

                # A: load x f32 on two parallel queues (sync for even groups,
                # gpsimd SWDGE for odd), DVE cast to bf16, then bf16 PE
                # transposes into xT
                for g in range(NB // 4):
                    xf = xs_pool.tile([128, 4 * D], F32, name="xf",
                                      tag="xf", bufs=2)
                    src = x_in[g * 512:(g + 1) * 512, :].rearrange(
                        "(t p) d -> p t d", p=128
                    )
                    eng = nc.sync if g % 2 == 0 else nc.gpsimd
                    eng.dma_start(xf.rearrange("p (t d) -> p t d", t=4), src)
                    xsb = xs_pool.tile([128, 4 * D], BF16, name="xsb",
                                       tag="xsb", bufs=2)
                    nc.vector.tensor_copy(xsb, xf)
                    for t in range(4):
                        nb = g * 4 + t
                        for dc in range(DC):
                            pst = tp_ps.tile([128, 128], BF16, name="pst", tag="pst")
                            nc.tensor.transpose(
                                pst, xsb[:, t * D + dc * 128: t * D + (dc + 1) * 128],
                                ident,
                            )
                            dst = xT[:, dc * N + nb * 128: dc * N + (nb + 1) * 128]
                            if (nb * DC + dc) % 2 == 0:
                                nc.vector.tensor_copy(dst, pst)
                            else:
                                nc.scalar.copy(dst, pst)

                # B: qkT -> kT (all), qT (own half)
                for nblk in range(N // 512):
                    ps = mm_ps.tile([128, 512], F32, name="qk_ps", tag="mmps")
                    for dc in range(DC):
                        nc.tensor.matmul(
                            ps,
                            wqk_sb[:, dc * E:(dc + 1) * E],
                            xT[:, dc * N + nblk * 512: dc * N + (nblk + 1) * 512],
                            start=(dc == 0),
                            stop=(dc == DC - 1),
                        )
                    nc.scalar.activation(
                        kT[:, nblk * 512:(nblk + 1) * 512], ps, AF.Identity,
                        bias=kb_sb, scale=kg_sb,
                    )
                    if nblk < NOWN // 512:
                        nc.scalar.activation(
                            qT[:, nblk * 512:(nblk + 1) * 512], ps, AF.Identity,
                            bias=qb_sb, scale=qg_sb,
                        )

                # C: v (natural layout)
                for jb in range(NB):
                    for hh in range(2):
                        ps = mm_ps.tile([128, 512], F32, name="v_ps", tag="mmps")
                        for dc in range(DC):
                            nc.tensor.matmul(
                                ps,
                                xT[:, dc * N + jb * 128: dc * N + (jb + 1) * 128],
                                wh_sb[:, dc * 2 * H + hh * 512: dc * 2 * H + (hh + 1) * 512],
                                start=(dc == 0),
                                stop=(dc == DC - 1),
                            )
                        dst = v_big[:, jb * H + hh * 512: jb * H + (hh + 1) * 512]
                        if (jb * 2 + hh) % 2 == 0:
                            nc.vector.tensor_copy(dst, ps)
                        else:
                            nc.scalar.copy(dst, ps)

                # C: gateT
                for hb in range(HB):
                    for ib in range(IBLK):
                        ps = mm_ps.tile([128, 512], F32, name="g_ps", tag="mmps")
                        for dc in range(DC):
                            nc.tensor.matmul(
                                ps,
                                wh_sb[:, dc * 2 * H + H + hb * 128: dc * 2 * H + H + (hb + 1) * 128],
                                xT[:, dc * N + ib * 512: dc * N + (ib + 1) * 512],
                                start=(dc == 0),
                                stop=(dc == DC - 1),
                            )
                        dst = gT_big[:, hb * NOWN + ib * 512: hb * NOWN + (ib + 1) * 512]
                        if (hb * IBLK + ib) % 2 == 0:
                            nc.vector.tensor_copy(dst, ps)
                        else:
                            nc.scalar.copy(dst, ps)

            # ---------------- phases D-F ----------------
            with tc.tile_pool(name="at_pool", bufs=36) as at_pool, \
                 tc.tile_pool(name="og_pool", bufs=10) as og_pool, \
                 tc.tile_pool(name="st_pool", bufs=2) as st_pool, \
                 tc.tile_pool(name="sim_ps", bufs=3, space="PSUM") as sim_ps, \
                 tc.tile_pool(name="ot_ps", bufs=2, space="PSUM") as ot_ps, \
                 tc.tile_pool(name="out_ps", bufs=2, space="PSUM") as out_ps:
                for blk in range(IBLK):
                    i0 = blk * 512
                    # D: simT -> attnT = relu(sim)^2, bf16
                    attn = []
                    for jc in range(JC):
                        sps = sim_ps.tile([128, 512], F32, name="sps", tag="sps")
                        nc.tensor.matmul(
                            sps,
                            kT[:, jc * 128:(jc + 1) * 128],
                            qT[:, i0:i0 + 512],
                            start=True, stop=True,
                        )
                        at = at_pool.tile([128, 512], BF16, name="at", tag="at")
                        nc.scalar.activation(at, sps, AF.Relu)
                        nc.vector.tensor_mul(at, at, at)
                        attn.append(at)

                    # E: oT chains (hc outer), gate multiply
                    og_tiles = []
                    for hc in range(HB):
                        ops = ot_ps.tile([128, 512], F32, name="ops", tag="ops")
                        for jc in range(JC):
                            nc.tensor.matmul(
                                ops,
                                v_big[:, jc * H + hc * 128: jc * H + (hc + 1) * 128],
                                attn[jc],
                                start=(jc == 0),
                                stop=(jc == JC - 1),
                            )
                        og = og_pool.tile([128, 512], BF16, name="og", tag="og")
                        nc.vector.tensor_mul(
                            og, ops, gT_big[:, hc * NOWN + i0: hc * NOWN + i0 + 512]
                        )
                        og_tiles.append(og)

                    # F: final projection + residual (merged per-block DMAs)
                    res = st_pool.tile([128, 4 * D], F32, name="res", tag="res")
                    nc.sync.dma_start(
                        res.rearrange("p (ic d) -> p ic d", ic=4),
                        x_in[i0:i0 + 512, :].rearrange("(ic p) d -> p ic d", p=128),
                    )
                    for ic in range(4):
                        ups = out_ps.tile([128, 512], F32, name="ups", tag="ups")
                        for hc in range(HB):
                            nc.tensor.matmul(
                                ups,
                                og_tiles[hc][:, ic * 128:(ic + 1) * 128],
                                wo_sb[:, hc * D:(hc + 1) * D],
                                start=(hc == 0),
                                stop=(hc == HB - 1),
                            )
                        osb = st_pool.tile([128, 512], F32, name="osb",
                                           tag="osb", bufs=6)
                        nc.vector.tensor_add(
                            osb, ups, res[:, ic * D:(ic + 1) * D]
                        )
                        row0 = i0 + ic * 128
                        nc.sync.dma_start(out[row0:row0 + 128, :], osb)
    nc.compile()
    return nc


def _numpy_fallback(x, w_hidden, b_hidden, w_qk, q_gamma, q_beta,
                    k_gamma, k_beta, w_out, b_out):
    scale = E ** -0.5
    hid = np.einsum("bnd,dh->bnh", x, w_hidden) + b_hidden
    v, gate = np.split(hid, 2, axis=-1)
    qk = np.einsum("bnd,de->bne", x, w_qk)
    q = qk * q_gamma + q_beta
    k = qk * k_gamma + k_beta
    sim = np.einsum("bid,bjd->bij", q, k) * scale
    attn = np.square(np.maximum(sim, 0.0))
    o = np.einsum("bij,bjh->bih", attn, v) * gate
    o = np.einsum("bnh,hd->bnd", o, w_out) + b_out
    return (o + x).astype(x.dtype)


def _run(inputs, trace=False):
    x = np.asarray(inputs["x"], dtype=np.float32)
    b_hidden = np.asarray(inputs["b_hidden"], dtype=np.float32)
    b_out = np.asarray(inputs["b_out"], dtype=np.float32)
    if np.any(b_hidden):
        # device kernel folds no hidden bias; this problem's setup has zeros
        return _numpy_fallback(**{k: np.asarray(v) for k, v in inputs.items()}), None

    bf16 = ml_dtypes.bfloat16
    s4 = float(E) ** -0.25  # sqrt of attention scale, folded into q and k
    wh_bf = np.ascontiguousarray(np.asarray(inputs["w_hidden"]).astype(bf16))
    wqk_bf = np.ascontiguousarray(np.asarray(inputs["w_qk"]).astype(bf16))
    wo_bf = np.ascontiguousarray(np.asarray(inputs["w_out"]).astype(bf16))
    qg2 = np.ascontiguousarray((np.asarray(inputs["q_gamma"]) * s4).astype(np.float32).reshape(E, 1))
    qb2 = np.ascontiguousarray((np.asarray(inputs["q_beta"]) * s4).astype(np.float32).reshape(E, 1))
    kg2 = np.ascontiguousarray((np.asarray(inputs["k_gamma"]) * s4).astype(np.float32).reshape(E, 1))
    kb2 = np.ascontiguousarray((np.asarray(inputs["k_beta"]) * s4).astype(np.float32).reshape(E, 1))

    if "nc" not in _CACHE:
        _CACHE["nc"] = build_nc()
    nc = _CACHE["nc"]

    in_maps = []
    for c in range(8):
        bi, hi = c // 2, c % 2
        xb = x[bi]
        xp = xb if hi == 0 else np.concatenate([xb[NOWN:], xb[:NOWN]], axis=0)
        in_maps.append({
            "x_in": np.ascontiguousarray(xp),
            "w_h": wh_bf, "w_qk": wqk_bf, "w_o": wo_bf,
            "qg": qg2, "qb": qb2, "kg": kg2, "kb": kb2,
        })

    kw = {}
    if trace:
        kw = dict(trace=True, trace_cores=[0])
    r = run_bass_kernel_spmd(nc, in_maps, core_ids=list(range(8)), **kw)

    out = np.empty((B, N, D), dtype=np.float32)
    for c in range(8):
        bi, hi = c // 2, c % 2
        out[bi, hi * NOWN:(hi + 1) * NOWN] = r.results[c]["out"]
    if np.any(b_out):
        out += b_out
    return out, r


def kernel(**inputs):
    out, _ = _run(inputs)
    return out
